# revision 18
# baseline (speedup 1.0000x reference)
"""Trainium2 Bass kernel for batched Jacobi iteration (5-point Laplacian).

Reference computation:
    x <- invD * (b - M x)   repeated `maxiter` times,
where M is the off-diagonal part of the 5-point Laplacian on a 512x512
grid, given in COO form.  For the actual inputs M is exactly the
4-neighbor stencil with value -1 and invD == 0.25, so the update is

    x_new[r, c] = 0.25 * (b[r, c] + x[r-1,c] + x[r+1,c] + x[r,c-1] + x[r,c+1])

(missing neighbors at grid edges contribute 0).

Strategy (8 NeuronCores, data parallel over batch B=16 -> 2 per core):
  - whole working set lives in SBUF for all iterations; ping-pong x
    buffers per batch; everything f32r so the PE streams 1 col/cycle
  - default layout 2: grid row r lives at (partition r//4, subrow r%4),
    stored [128, 4 subrows, 514 cols] with 1 zero pad col each side.
    N/S coupling = 6 in-partition identity matmuls (subrow-shifted
    moving APs) + 2 partition-coupling matmuls (pd/pu) per batch
  - PSUM accumulates 0.25*(N + S + b + E) via TensorE (E-neighbor =
    identity stationary with column-shifted moving AP, e_on_pe planes)
  - one DVE scalar_tensor_tensor finishes each plane group:
      x_new = 0.25 * x_W + psum        (west neighbor + combine + writeback)
    planes not in e_on_pe get E via an explicit DVE add instead (engine
    balance knob; measured best: e_on_pe=(0,1,2))
  - the whole solve (input DMA, iterations, output DMA) sits inside a
    hardware For_i(0, reps) loop so timing can amplify device execution
    without growing the program
"""

import sys

sys.path.insert(0, "/opt/trn_rl_repo")

import numpy as np

_N = 512  # grid side
_PL = 4  # row planes per grid
_P = 128  # partitions
_W = _N + 2  # padded row width (1 zero col each side)
_NCORES = 8
_BPC = 2  # batches per core

# planes whose E-neighbor term is computed on the TensorE (identity matmul
# with shifted moving AP); the rest go through an extra DVE add.  Tunable
# engine-balance knob.
_E_ON_PE = (0, 1, 2, 3)


def _build_nc(maxiter: int, reps: int, e_on_pe: tuple = None):
    import concourse.bacc as bacc
    import concourse.mybir as mybir
    from concourse.tile import TileContext

    f32 = mybir.dt.float32
    f32r = mybir.dt.float32r
    nc = bacc.Bacc("TRN2", target_bir_lowering=False, debug=False, num_devices=_NCORES)

    # everything f32r end-to-end: same bits as fp32 on the host, but the
    # PE streams it at 1 col/cycle (plain fp32 matmul is 4x slower) and the
    # BIR verifier demands f32r consumers see f32r producers
    u_in = nc.declare_dram_parameter("u", [_BPC, _PL, _P, _N], f32r, isOutput=False)
    b_in = nc.declare_dram_parameter("b", [_BPC, _PL, _P, _N], f32r, isOutput=False)
    tm_in = nc.declare_dram_parameter("tm", [_P, _P], f32r, isOutput=False)
    cn_in = nc.declare_dram_parameter("cn", [_P, _P], f32r, isOutput=False)
    cs_in = nc.declare_dram_parameter("cs", [_P, _P], f32r, isOutput=False)
    im_in = nc.declare_dram_parameter("im", [_P, _P], f32r, isOutput=False)
    out = nc.declare_dram_parameter("out", [_BPC, _PL, _P, _N], f32r, isOutput=True)

    assert maxiter % 2 == 0, "ping-pong buffers need an even iteration count"
    if e_on_pe is None:
        e_on_pe = _E_ON_PE
    # DVE-handled planes must be one contiguous block for clean slicing
    dve_planes = tuple(g for g in range(_PL) if g not in e_on_pe)
    if dve_planes:
        lo, hi = dve_planes[0], dve_planes[-1] + 1
        assert dve_planes == tuple(range(lo, hi))
    pe_planes = tuple(g for g in range(_PL) if g in e_on_pe)
    if pe_planes:
        plo, phi = pe_planes[0], pe_planes[-1] + 1
        assert pe_planes == tuple(range(plo, phi))

    with TileContext(nc) as tc:
        with (
            tc.tile_pool(name="const", bufs=1) as const,
            tc.tile_pool(name="state", bufs=1) as state,
            tc.tile_pool(name="psum", bufs=2, space="PSUM") as psum,
        ):
            tm = const.tile([_P, _P], f32r, tag="tm")
            cn = const.tile([_P, _P], f32r, tag="cn")
            cs = const.tile([_P, _P], f32r, tag="cs")
            im = const.tile([_P, _P], f32r, tag="im")
            nc.sync.dma_start(tm[:], tm_in[:])
            nc.sync.dma_start(cn[:], cn_in[:])
            nc.sync.dma_start(cs[:], cs_in[:])
            nc.sync.dma_start(im[:], im_in[:])

            xa, xb, bts, ts = [], [], [], []
            for bi in range(_BPC):
                x0 = state.tile([_P, _PL, _W], f32r, tag=f"xa{bi}")
                x1 = state.tile([_P, _PL, _W], f32r, tag=f"xb{bi}")
                bt = state.tile([_P, _PL, _N], f32r, tag=f"b{bi}")
                if dve_planes:
                    tt = state.tile([_P, len(dve_planes), _N], f32, tag=f"t{bi}")
                    ts.append(tt)
                # zero once so pad columns stay zero forever (interior
                # rewrites never touch them); memset rejects f32r, so bitcast
                nc.gpsimd.memset(x0[:].bitcast(f32), 0.0)
                nc.gpsimd.memset(x1[:].bitcast(f32), 0.0)
                xa.append(x0)
                xb.append(x1)
                bts.append(bt)

            with tc.For_i(0, reps, name="rep"):
                for bi in range(_BPC):
                    for g in range(_PL):
                        nc.sync.dma_start(xa[bi][:, g, 1 : 1 + _N], u_in[bi, g])
                        nc.sync.dma_start(bts[bi][:, g, :], b_in[bi, g])

                for it in range(maxiter):
                    src, dst = (xa, xb) if it % 2 == 0 else (xb, xa)
                    for bi in range(_BPC):
                        x = src[bi]
                        p = psum.tile([_P, _PL, _N], f32, tag="p")
                        for g in range(_PL):
                            mms = [
                                (tm, x[:, g, 1 : 1 + _N]),
                                (im, bts[bi][:, g, :]),
                            ]
                            if g in e_on_pe:
                                mms.append((im, x[:, g, 2 : 2 + _N]))
                            if g > 0:
                                mms.append((cn, x[:, g - 1, 1 : 1 + _N]))
                            if g < _PL - 1:
                                mms.append((cs, x[:, g + 1, 1 : 1 + _N]))
                            for i, (mat, rhs) in enumerate(mms):
                                nc.tensor.matmul(
                                    p[:, g, :],
                                    mat[:],
                                    rhs,
                                    start=(i == 0),
                                    stop=(i == len(mms) - 1),
                                )
                        # x_new = 0.25 * x_W + psum   (W fused into the
                        # combine; E came via PSUM for e_on_pe planes, via
                        # the explicit t add for the rest)
                        if dve_planes:
                            t = ts[bi]
                            nc.vector.tensor_add(
                                t[:],
                                x[:, lo:hi, 0:_N],
                                x[:, lo:hi, 2 : 2 + _N],
                            )
                            nc.vector.scalar_tensor_tensor(
                                dst[bi][:, lo:hi, 1 : 1 + _N],
                                t[:],
                                0.25,
                                p[:, lo:hi, :],
                                mybir.AluOpType.mult,
                                mybir.AluOpType.add,
                            )
                        if pe_planes:
                            nc.vector.scalar_tensor_tensor(
                                dst[bi][:, plo:phi, 1 : 1 + _N],
                                x[:, plo:phi, 0:_N],
                                0.25,
                                p[:, plo:phi, :],
                                mybir.AluOpType.mult,
                                mybir.AluOpType.add,
                            )

                for bi in range(_BPC):
                    for g in range(_PL):
                        nc.sync.dma_start(out[bi, g], xa[bi][:, g, 1 : 1 + _N])

    nc.finalize()
    return nc


def _build_nc2(maxiter: int, reps: int, e_on_pe: tuple = None):
    """Layout 2: grid row r lives at (partition r//4, subrow r%4).

    N/S coupling then needs only 6 in-partition identity matmuls with
    subrow-shifted moving APs plus 2 partition-coupling matmuls (pd/pu),
    1024 fewer PE columns per batch-iteration than layout 1, and u/b/out
    transfer as one contiguous DMA per batch.
    """
    import concourse.bacc as bacc
    import concourse.mybir as mybir
    from concourse.tile import TileContext

    f32 = mybir.dt.float32
    f32r = mybir.dt.float32r
    nc = bacc.Bacc("TRN2", target_bir_lowering=False, debug=False, num_devices=_NCORES)

    u_in = nc.declare_dram_parameter("u", [_BPC, _P, _PL, _N], f32r, isOutput=False)
    b_in = nc.declare_dram_parameter("b", [_BPC, _P, _PL, _N], f32r, isOutput=False)
    pd_in = nc.declare_dram_parameter("pd", [_P, _P], f32r, isOutput=False)
    pu_in = nc.declare_dram_parameter("pu", [_P, _P], f32r, isOutput=False)
    im_in = nc.declare_dram_parameter("im", [_P, _P], f32r, isOutput=False)
    out = nc.declare_dram_parameter("out", [_BPC, _P, _PL, _N], f32r, isOutput=True)

    assert maxiter % 2 == 0, "ping-pong buffers need an even iteration count"
    if e_on_pe is None:
        e_on_pe = _E_ON_PE
    # e_on_pe: one tuple applied to both batch streams, or a pair of
    # tuples (one per batch) for asymmetric engine balance
    if e_on_pe and isinstance(e_on_pe[0], tuple):
        per_batch = e_on_pe
    else:
        per_batch = (e_on_pe,) * _BPC
    splits = []
    for eb in per_batch:
        dve_pl = tuple(g for g in range(_PL) if g not in eb)
        pe_pl = tuple(g for g in range(_PL) if g in eb)
        if dve_pl:
            assert dve_pl == tuple(range(dve_pl[0], dve_pl[-1] + 1))
        if pe_pl:
            assert pe_pl == tuple(range(pe_pl[0], pe_pl[-1] + 1))
        splits.append((eb, dve_pl, pe_pl))

    with TileContext(nc) as tc:
        with (
            tc.tile_pool(name="const", bufs=1) as const,
            tc.tile_pool(name="state", bufs=1) as state,
            tc.tile_pool(name="psum", bufs=2, space="PSUM") as psum,
        ):
            pd = const.tile([_P, _P], f32r, tag="pd")
            pu = const.tile([_P, _P], f32r, tag="pu")
            im = const.tile([_P, _P], f32r, tag="im")
            nc.sync.dma_start(pd[:], pd_in[:])
            nc.sync.dma_start(pu[:], pu_in[:])
            nc.sync.dma_start(im[:], im_in[:])

            xa, xb, bts, ts = [], [], [], []
            for bi in range(_BPC):
                x0 = state.tile([_P, _PL, _W], f32r, tag=f"xa{bi}")
                x1 = state.tile([_P, _PL, _W], f32r, tag=f"xb{bi}")
                bt = state.tile([_P, _PL, _N], f32r, tag=f"b{bi}")
                if splits[bi][1]:
                    tt = state.tile(
                        [_P, len(splits[bi][1]), _N], f32, tag=f"t{bi}"
                    )
                    ts.append(tt)
                else:
                    ts.append(None)
                nc.gpsimd.memset(x0[:].bitcast(f32), 0.0)
                nc.gpsimd.memset(x1[:].bitcast(f32), 0.0)
                xa.append(x0)
                xb.append(x1)
                bts.append(bt)

            with tc.For_i(0, reps, name="rep"):
                for bi in range(_BPC):
                    nc.sync.dma_start(xa[bi][:, :, 1 : 1 + _N], u_in[bi])
                    nc.sync.dma_start(bts[bi][:, :, :], b_in[bi])

                for it in range(maxiter):
                    src, dst = (xa, xb) if it % 2 == 0 else (xb, xa)
                    for bi in range(_BPC):
                        eb, dve_planes, pe_planes = splits[bi]
                        x = src[bi]
                        p = psum.tile([_P, _PL, _N], f32, tag="p")
                        for s in range(_PL):
                            # N neighbor: row 4p+s-1
                            if s == 0:
                                mms = [(pd, x[:, _PL - 1, 1 : 1 + _N])]
                            else:
                                mms = [(im, x[:, s - 1, 1 : 1 + _N])]
                            # S neighbor: row 4p+s+1
                            if s == _PL - 1:
                                mms.append((pu, x[:, 0, 1 : 1 + _N]))
                            else:
                                mms.append((im, x[:, s + 1, 1 : 1 + _N]))
                            mms.append((im, bts[bi][:, s, :]))
                            if s in eb:
                                mms.append((im, x[:, s, 2 : 2 + _N]))
                            for i, (mat, rhs) in enumerate(mms):
                                nc.tensor.matmul(
                                    p[:, s, :],
                                    mat[:],
                                    rhs,
                                    start=(i == 0),
                                    stop=(i == len(mms) - 1),
                                )
                        if dve_planes:
                            lo, hi = dve_planes[0], dve_planes[-1] + 1
                            t = ts[bi]
                            nc.vector.tensor_add(
                                t[:],
                                x[:, lo:hi, 0:_N],
                                x[:, lo:hi, 2 : 2 + _N],
                            )
                            nc.vector.scalar_tensor_tensor(
                                dst[bi][:, lo:hi, 1 : 1 + _N],
                                t[:],
                                0.25,
                                p[:, lo:hi, :],
                                mybir.AluOpType.mult,
                                mybir.AluOpType.add,
                            )
                        if pe_planes:
                            plo, phi = pe_planes[0], pe_planes[-1] + 1
                            nc.vector.scalar_tensor_tensor(
                                dst[bi][:, plo:phi, 1 : 1 + _N],
                                x[:, plo:phi, 0:_N],
                                0.25,
                                p[:, plo:phi, :],
                                mybir.AluOpType.mult,
                                mybir.AluOpType.add,
                            )

                for bi in range(_BPC):
                    nc.sync.dma_start(out[bi], xa[bi][:, :, 1 : 1 + _N])

    nc.finalize()
    return nc


def _stencil_mats2():
    # layout 2 stationaries, pre-scaled by 0.25.  row r = 4p + s.
    s_ = 0.25
    idx = np.arange(_P - 1)
    pd = np.zeros((_P, _P), np.float32)
    pd[idx, idx + 1] = s_  # x[p-1, 3] -> out[p, 0]
    pu = np.zeros((_P, _P), np.float32)
    pu[idx + 1, idx] = s_  # x[p+1, 0] -> out[p, 3]
    im = s_ * np.eye(_P, dtype=np.float32)
    return pd, pu, im


_NC_CACHE: dict = {}


def _get_nc(maxiter: int, reps: int = 1, e_on_pe: tuple = None, layout: int = 1):
    key = (maxiter, reps, e_on_pe, layout)
    if key not in _NC_CACHE:
        build = _build_nc if layout == 1 else _build_nc2
        _NC_CACHE[key] = build(maxiter, reps, e_on_pe)
    return _NC_CACHE[key]


def _stencil_mats():
    # all stationaries pre-scaled by 0.25 so PSUM directly accumulates
    # 0.25*(b + xN + xS + xE)
    s = 0.25
    tm = np.zeros((_P, _P), np.float32)
    idx = np.arange(_P - 1)
    tm[idx, idx + 1] = s  # contribution of x[k] to out[k+1] (south nbr of k)
    tm[idx + 1, idx] = s  # north
    cn = np.zeros((_P, _P), np.float32)
    cn[_P - 1, 0] = s  # plane g-1 row 127 -> plane g row 0
    cs = np.zeros((_P, _P), np.float32)
    cs[0, _P - 1] = s  # plane g+1 row 0 -> plane g row 127
    im = s * np.eye(_P, dtype=np.float32)
    return tm, cn, cs, im


def _expected_stencil():
    # same construction as the reference's _stencil_offdiag
    g = np.arange(_N * _N, dtype=np.int32).reshape(_N, _N)
    rows = np.concatenate(
        [g[:, :-1].ravel(), g[:, 1:].ravel(), g[:-1, :].ravel(), g[1:, :].ravel()]
    )
    cols = np.concatenate(
        [g[:, 1:].ravel(), g[:, :-1].ravel(), g[1:, :].ravel(), g[:-1, :].ravel()]
    )
    return rows, cols


def _verify_stencil(M_rows, M_cols, M_vals, invD):
    """Check the COO matrix is exactly the uniform -1 4-neighbor stencil
    (no wraps) and invD == 0.25 everywhere."""
    r = np.asarray(M_rows)
    c = np.asarray(M_cols)
    v = np.asarray(M_vals)
    if not (np.all(np.asarray(invD) == np.float32(0.25)) and np.all(v == np.float32(-1.0))):
        return False
    er, ec = _expected_stencil()
    if r.shape == er.shape and np.array_equal(r, er) and np.array_equal(c, ec):
        return True  # fast path: byte-identical to the reference construction
    # thorough order-independent check
    r = r.astype(np.int64)
    c = c.astype(np.int64)
    off = c - r
    n2 = _N * _N
    bands = {o: off == o for o in (1, -1, _N, -_N)}
    if not (bands[1] | bands[-1] | bands[_N] | bands[-_N]).all():
        return False
    if np.any((r[bands[1]] % _N) == _N - 1) or np.any((r[bands[-1]] % _N) == 0):
        return False
    rows2 = np.arange(n2)
    for o, m in bands.items():
        cnt = np.bincount(r[m], minlength=n2)
        if o == 1:
            want = (rows2 % _N) != _N - 1
        elif o == -1:
            want = (rows2 % _N) != 0
        elif o == _N:
            want = rows2 < n2 - _N
        else:
            want = rows2 >= _N
        if not np.array_equal(cnt, want.astype(cnt.dtype)):
            return False
    return True


def _fallback(u, b, M_rows, M_cols, M_vals, invD, maxiter):
    """Host scipy path — only taken if inputs are not the expected stencil."""
    from scipy.sparse import coo_matrix

    Bn = u.shape[0]
    n2 = _N * _N
    M = coo_matrix(
        (np.asarray(M_vals), (np.asarray(M_rows), np.asarray(M_cols))),
        shape=(n2, n2),
    ).tocsr()
    x = np.asarray(u).reshape(Bn, -1).astype(np.float32)
    bb = np.asarray(b).astype(np.float32)
    iD = np.asarray(invD).astype(np.float32)
    for _ in range(int(maxiter)):
        x = ((bb - (M @ x.T).T) * iD[None, :]).astype(np.float32)
    return x.reshape(u.shape)


class _CachedRunner:
    """Reusable jitted PJRT executor for one Bass module (axon path).

    Mirrors concourse.bass2jax.run_bass_via_pjrt but caches the jitted
    callable so repeated calls skip retrace / executable rebuild.
    """

    def __init__(self, nc, n_cores):
        import jax
        from jax.sharding import Mesh, PartitionSpec
        from jax.experimental.shard_map import shard_map
        import concourse.mybir as mybir
        from concourse.bass2jax import (
            _bass_exec_p,
            install_neuronx_cc_hook,
            partition_id_tensor,
        )

        install_neuronx_cc_hook()
        assert nc.dbg_addr is None
        self.n_cores = n_cores

        partition_name = (
            nc.partition_id_tensor.name if nc.partition_id_tensor else None
        )
        in_names, out_names, out_avals, zero_outs = [], [], [], []
        for alloc in nc.m.functions[0].allocations:
            if not isinstance(alloc, mybir.MemoryLocationSet):
                continue
            name = alloc.memorylocations[0].name
            if alloc.kind == "ExternalInput":
                if name != partition_name:
                    in_names.append(name)
            elif alloc.kind == "ExternalOutput":
                out_names.append(name)
                shape = tuple(alloc.tensor_shape)
                dtype = mybir.dt.np(alloc.dtype)
                out_avals.append(jax.core.ShapedArray(shape, dtype))
                zero_outs.append(np.zeros(shape, dtype))
        self.in_names = in_names
        self.out_names = out_names
        self.out_avals = out_avals
        n_params = len(in_names)
        n_outs = len(out_avals)
        all_in_names = list(in_names) + list(out_names)
        if partition_name is not None:
            all_in_names.append(partition_name)
        donate = tuple(range(n_params, n_params + n_outs))

        def _body(*args):
            operands = list(args)
            if partition_name is not None:
                operands.append(partition_id_tensor())
            outs = _bass_exec_p.bind(
                *operands,
                out_avals=tuple(out_avals),
                in_names=tuple(all_in_names),
                out_names=tuple(out_names),
                lowering_input_output_aliases=(),
                sim_require_finite=True,
                sim_require_nnan=True,
                nc=nc,
            )
            return tuple(outs)

        devices = jax.devices()[:n_cores]
        assert len(devices) == n_cores
        mesh = Mesh(np.asarray(devices), ("core",))
        in_specs = (PartitionSpec("core"),) * (n_params + n_outs)
        out_specs = (PartitionSpec("core"),) * len(out_names)
        self._sharded = jax.jit(
            shard_map(
                _body,
                mesh=mesh,
                in_specs=in_specs,
                out_specs=out_specs,
                check_rep=False,
            ),
            donate_argnums=donate,
            keep_unused=True,
        )
        self._concat_zeros = [
            np.zeros((n_cores * z.shape[0], *z.shape[1:]), z.dtype)
            for z in zero_outs
        ]

    def __call__(self, in_maps):
        n_cores = self.n_cores
        concat_in = [
            np.concatenate(
                [np.asarray(in_maps[c][name]) for c in range(n_cores)], axis=0
            )
            for name in self.in_names
        ]
        out_arrs = self._sharded(*concat_in, *self._concat_zeros)
        return [
            {
                name: np.asarray(out_arrs[i]).reshape(
                    n_cores, *self.out_avals[i].shape
                )[c]
                for i, name in enumerate(self.out_names)
            }
            for c in range(n_cores)
        ]


_RUNNER_CACHE: dict = {}


def _get_runner(maxiter: int, reps: int = 1, e_on_pe: tuple = None, layout: int = 1):
    key = (maxiter, reps, e_on_pe, layout)
    if key not in _RUNNER_CACHE:
        _RUNNER_CACHE[key] = _CachedRunner(
            _get_nc(maxiter, reps, e_on_pe, layout), _NCORES
        )
    return _RUNNER_CACHE[key]


def _make_in_maps(u, b, layout: int = 1):
    Bn = u.shape[0]
    assert Bn == _NCORES * _BPC
    if layout == 1:
        consts = dict(zip(("tm", "cn", "cs", "im"), _stencil_mats()))
        u4 = np.ascontiguousarray(u.reshape(Bn, _PL, _P, _N), dtype=np.float32)
        b4 = np.ascontiguousarray(b.reshape(Bn, _PL, _P, _N), dtype=np.float32)
    else:
        consts = dict(zip(("pd", "pu", "im"), _stencil_mats2()))
        u4 = np.ascontiguousarray(u.reshape(Bn, _P, _PL, _N), dtype=np.float32)
        b4 = np.ascontiguousarray(b.reshape(Bn, _P, _PL, _N), dtype=np.float32)
    in_maps = []
    for k in range(_NCORES):
        in_maps.append(
            {
                "u": u4[_BPC * k : _BPC * (k + 1)],
                "b": b4[_BPC * k : _BPC * (k + 1)],
                **consts,
            }
        )
    return in_maps


# active configuration: (e_on_pe, layout) — both out tensor layouts flatten
# back to grid order with a plain reshape
_CONFIG = {"e_on_pe": (0, 1, 2), "layout": 2}


def kernel(u, b, M_rows, M_cols, M_vals, invD, maxiter):
    u = np.asarray(u)
    b = np.asarray(b)
    mi = int(maxiter)

    if mi % 2 != 0 or not _verify_stencil(M_rows, M_cols, M_vals, invD):
        return _fallback(u, b, M_rows, M_cols, M_vals, invD, maxiter)

    run = _get_runner(mi, 1, _CONFIG["e_on_pe"], _CONFIG["layout"])
    res = run(_make_in_maps(u, b, _CONFIG["layout"]))
    outs = [res[k]["out"] for k in range(_NCORES)]
    full = np.concatenate(outs, axis=0).reshape(u.shape).astype(np.float32)
    return full
